# revision 41
# baseline (speedup 1.0000x reference)
"""CASS block (LayerNorm + gradient-selected scan + fc1/dwconv/gelu/fc2 + residual)
on 8 TRN2 NeuronCores, pure data parallel over the batch.

Key algebra: with per-pixel LN stats (mu, rstd) and per-channel (gamma, beta),
    u = LN(x) @ W1 + b1 = rstd * (x @ (gamma*W1) - mu (x) colsum(gamma*W1)) + (beta@W1 + b1)
The kernel computes psum = x @ W1g - mu (x) s1g on the TensorEngine by appending
mu as one extra contraction row (written at runtime into the channel-major input),
then multiplies by a partition-broadcast rstd row during PSUM evacuation.  The
fc1 bias b1'' is deferred through the (linear) depthwise conv into the Gelu
bias, with two boundary-column fixups for the zero-padded conv ends.  fc2's bias
is accumulated into PSUM by a K=1 ones-row matmul before the fc2 matmuls, and
fc2 uses the gelu output as the stationary operand so the result comes out
pixel-major: the residual add + store need no back-transpose.

The channel-major bf16 copy of x is built host-side during sharding (pure input
layout change); stats/residual still use the natural pixel-major fp32 x.

The gradient selector: for uniform gamma the "gray" image mean_c(LN(x)) is a
constant, so grad_h = grad_v = 0, the MLP logits tie, softmax gives exactly
0.25 each in fp32, and argmax -> idx 0 for every sample: the 'v' (transpose)
branch is dead.  The device kernel therefore always scans row-major; a host
fallback handles non-uniform gamma by pre-transposing flagged samples (the
row-major reshape of the result is orientation-identical, so y_ref = x + (y_dev
- x_dev) recovers the reference output exactly).
"""

import numpy as np
import ml_dtypes

import concourse.bass as bass
import concourse.mybir as mybir
import concourse.tile as tile
from concourse import bacc

B, H, W, C = 32, 56, 56, 192
D = 384                      # D_INNER
NCORES = 8
S = B // NCORES              # samples per core
L = H * W                    # 3136 pixels per sample
PT = 128                     # pixels per partition tile
NT = (L + PT - 1) // PT      # 25 pixel tiles (24 full + 64 tail)
TAIL = L - (NT - 1) * PT     # 64
NBLK = 7                     # fc1 N blocks per sample
NB = L // NBLK               # 448 columns per fc1 matmul
EPS = 1e-5
F32 = mybir.dt.float32
BF16 = mybir.dt.bfloat16
AL = mybir.AluOpType
AF = mybir.ActivationFunctionType

_CACHE = {}


def _build_nc():
    nc = bacc.Bacc()
    x_d = nc.declare_dram_parameter("x", [S * L, C], F32, isOutput=False)
    xb_d = nc.declare_dram_parameter("xb", [S * L, C], F32, isOutput=False)
    xt_d = nc.declare_dram_parameter("xt", [S, 128, 2, L], BF16, isOutput=False)
    w1a_d = nc.declare_dram_parameter("w1a", [128, D], BF16, isOutput=False)
    w1b_d = nc.declare_dram_parameter("w1b", [65, D], BF16, isOutput=False)
    w2_d = nc.declare_dram_parameter("w2", [128, 3, C], BF16, isOutput=False)
    dwk_d = nc.declare_dram_parameter("dwk", [128, 3, 3], F32, isOutput=False)
    gb_d = nc.declare_dram_parameter("gb", [128, 3], F32, isOutput=False)
    bw_d = nc.declare_dram_parameter("bw", [128, 3, 2], F32, isOutput=False)
    y_d = nc.declare_dram_parameter("y", [S * L, C], F32, isOutput=True)

    with tile.TileContext(nc) as tc, \
         tc.tile_pool(name="const", bufs=1) as const, \
         tc.tile_pool(name="xpool", bufs=6) as xpool, \
         tc.tile_pool(name="xt", bufs=4) as xtpool, \
         tc.tile_pool(name="stat", bufs=2) as stat, \
         tc.tile_pool(name="rb", bufs=4) as rbpool, \
         tc.tile_pool(name="u", bufs=2) as upool, \
         tc.tile_pool(name="t", bufs=2) as tpool, \
         tc.tile_pool(name="s2", bufs=2) as s2pool, \
         tc.tile_pool(name="y", bufs=4) as ypool, \
         tc.tile_pool(name="pf1", bufs=5, space="PSUM") as pf1, \
         tc.tile_pool(name="pf2", bufs=3, space="PSUM") as pf2:

        w1a = const.tile([128, D], BF16)
        w1b = const.tile([65, D], BF16)
        w2 = const.tile([128, 3, C], BF16)
        dwk = const.tile([128, 3, 3], F32)
        gb = const.tile([128, 3], F32)
        bw = const.tile([128, 3, 2], F32)
        eps_sb = const.tile([128, 1], F32)
        nc.sync.dma_start(out=w1a, in_=w1a_d[:, :])
        nc.sync.dma_start(out=w1b, in_=w1b_d[:, :])
        nc.sync.dma_start(out=w2, in_=w2_d[:, :, :])
        nc.sync.dma_start(out=dwk, in_=dwk_d[:, :, :])
        nc.sync.dma_start(out=gb, in_=gb_d[:, :])
        nc.sync.dma_start(out=bw, in_=bw_d[:, :, :])
        nc.vector.memset(eps_sb, EPS)

        state = {}

        def pre_sample(s):
            base = s * L

            # ---- stats: stream x (pixel-major fp32) through small chunks on
            #      the SWDGE queues; x is dead after bn_stats.  The
            #      channel-major bf16 copy (fc1 rhs) loads here too so the mu
            #      row can be linearized straight into it.
            xt = xtpool.tile([128, 2, L], BF16)
            nc.sync.dma_start(out=xt, in_=xt_d[s, :, :, :])

            # ---- LN stats -> pack[:,0,k]=mu_k, pack[:,1,k]=var_k->rstd_k
            bns = stat.tile([128, NT, 6], F32)
            pack = stat.tile([128, 2, 64], F32)
            packb = stat.tile([128, 2, 64], BF16)
            nc.vector.memset(pack, 0.0)
            for j in range(12):
                x_sb = xpool.tile([128, 2, C], F32)
                nc.gpsimd.dma_start(
                    out=x_sb,
                    in_=x_d[base + j * 256: base + (j + 1) * 256, :]
                        .rearrange("(two p) c -> p two c", p=128),
                )
                for jj in range(2):
                    k = 2 * j + jj
                    nc.vector.bn_stats(out=bns[:, k:k + 1, :],
                                       in_=x_sb[:, jj:jj + 1, :])
                    nc.vector.bn_aggr(out=pack[:, :, k],
                                      in_=bns[:, k:k + 1, :])
            x_sb = xpool.tile([128, 2, C], F32)
            nc.gpsimd.dma_start(
                out=x_sb[0:TAIL, 0, :],
                in_=x_d[base + (NT - 1) * PT: base + L, :],
            )
            nc.vector.bn_stats(out=bns[0:TAIL, NT - 1:NT, :],
                               in_=x_sb[0:TAIL, 0:1, :])
            nc.vector.bn_aggr(out=pack[0:TAIL, :, NT - 1],
                              in_=bns[0:TAIL, NT - 1:NT, :])

            # rstd = 1/sqrt(var+eps) in place
            nc.scalar.activation(out=pack[:, 1, 0:NT], in_=pack[:, 1, 0:NT],
                                 func=AF.Sqrt, bias=eps_sb[:, :], scale=1.0)
            nc.vector.reciprocal(out=pack[:, 1, 0:NT], in_=pack[:, 1, 0:NT])
            nc.vector.tensor_copy(out=packb, in_=pack)

            # ---- one small xbar transpose + row-linearize DMAs:
            #      mu row -> xt[64,1,:] (fc1 aug row), rstd row -> broadcast
            packT = stat.tile([128, 128], BF16)
            nc.sync.dma_start(out=packT,
                              in_=packb.rearrange("p a b -> p (a b)"),
                              transpose=True)
            nc.sync.dma_start(out=xt[64:65, 1, 0:(NT - 1) * PT],
                              in_=packT[0:NT - 1, :])
            nc.sync.dma_start(out=xt[64:65, 1, (NT - 1) * PT:L],
                              in_=packT[NT - 1:NT, 0:TAIL])
            rrow = stat.tile([1, L], BF16)
            nc.sync.dma_start(out=rrow[0:1, 0:(NT - 1) * PT],
                              in_=packT[64:64 + NT - 1, :])
            nc.sync.dma_start(out=rrow[0:1, (NT - 1) * PT:L],
                              in_=packT[64 + NT - 1:64 + NT, 0:TAIL])
            rstd_b = rbpool.tile([128, L], BF16)
            nc.gpsimd.partition_broadcast(rstd_b, rrow[0:1, :])
            state[s] = (xt, rstd_b)

        def main_sample(s):
            base = s * L
            xt, rstd_b = state.pop(s)

            # ---- fc1 (+ LN fold): psum = x@W1g - mu(x)s1g; u = rstd * psum
            u = upool.tile([128, 3, L + 2], BF16)
            nc.vector.memset(u[:, :, 0:1], 0.0)
            nc.vector.memset(u[:, :, L + 1:L + 2], 0.0)
            for blk in range(NBLK):
                cs = blk * NB
                for m in range(3):
                    pt_ = pf1.tile([128, NB], F32)
                    nc.tensor.matmul(pt_, lhsT=w1a[:, m * 128:(m + 1) * 128],
                                     rhs=xt[:, 0, cs:cs + NB],
                                     start=True, stop=False)
                    nc.tensor.matmul(pt_, lhsT=w1b[:, m * 128:(m + 1) * 128],
                                     rhs=xt[0:65, 1, cs:cs + NB],
                                     start=False, stop=True)
                    dst = u[:, m, 1 + cs:1 + cs + NB]
                    nc.scalar.copy(out=dst, in_=pt_)
                    nc.vector.tensor_tensor(out=dst, in0=dst,
                                            in1=rstd_b[:, cs:cs + NB],
                                            op=AL.mult)

            # ---- depthwise 3-tap conv along the scan; biases fold into the
            #      gelu bias (gb), with 2 boundary fixups (bw)
            t = tpool.tile([128, 3, L], BF16)
            LH = L // 2
            for m in range(3):
                for h in range(2):
                    lo, hi = h * LH, (h + 1) * LH
                    sa = s2pool.tile([128, LH], BF16, tag="sa")
                    sc = s2pool.tile([128, LH], BF16, tag="sc")
                    nc.scalar.activation(out=sa, in_=u[:, m, lo:hi],
                                         func=AF.Copy, bias=0.0,
                                         scale=dwk[:, m, 0:1])
                    nc.vector.tensor_scalar(out=sc, in0=u[:, m, lo + 2:hi + 2],
                                            scalar1=dwk[:, m, 2:3],
                                            scalar2=None, op0=AL.mult)
                    nc.vector.tensor_scalar(out=t[:, m, lo:hi],
                                            in0=u[:, m, lo + 1:hi + 1],
                                            scalar1=dwk[:, m, 1:2],
                                            scalar2=None, op0=AL.mult)
                    nc.vector.tensor_tensor(out=t[:, m, lo:hi],
                                            in0=t[:, m, lo:hi],
                                            in1=sa, op=AL.add)
                    nc.vector.tensor_tensor(out=t[:, m, lo:hi],
                                            in0=t[:, m, lo:hi],
                                            in1=sc, op=AL.add)
                    if h == 0:
                        nc.vector.tensor_scalar(out=t[:, m, 0:1],
                                                in0=t[:, m, 0:1],
                                                scalar1=bw[:, m, 0:1],
                                                scalar2=None, op0=AL.subtract)
                    else:
                        nc.vector.tensor_scalar(out=t[:, m, L - 1:L],
                                                in0=t[:, m, L - 1:L],
                                                scalar1=bw[:, m, 1:2],
                                                scalar2=None, op0=AL.subtract)
                    nc.scalar.activation(out=t[:, m, lo:hi],
                                         in_=t[:, m, lo:hi],
                                         func=AF.Gelu, bias=gb[:, m:m + 1],
                                         scale=1.0)

            # ---- fc2: psum preloaded with fc2_b via K=1 ones matmul, then
            #      g as stationary operand -> pixel-major out; +x residual
            for k in range(NT):
                psz = PT if k < NT - 1 else TAIL
                xb_sb = ypool.tile([128, C], F32, tag="xb")
                nc.sync.dma_start(
                    out=xb_sb[:psz, :],
                    in_=xb_d[base + k * PT: base + k * PT + psz, :])
                py = pf2.tile([128, C], F32)
                for kc in range(3):
                    nc.tensor.matmul(py[:psz, :],
                                     lhsT=t[:, kc, k * PT:k * PT + psz],
                                     rhs=w2[:, kc, :],
                                     start=(kc == 0), stop=(kc == 2))
                y_sb = ypool.tile([128, C], F32)
                nc.vector.tensor_tensor(out=y_sb[:psz, :], in0=py[:psz, :],
                                        in1=xb_sb[:psz, :], op=AL.add)
                nc.sync.dma_start(out=y_d[base + k * PT: base + k * PT + psz, :],
                                  in_=y_sb[:psz, :])

        # all stats/rstd prep up front (x is dead afterwards); the main
        # pipeline is then stats-free so PE can run ahead across samples
        for s in range(S):
            pre_sample(s)
        for s in range(S):
            main_sample(s)
    nc.finalize()
    return nc


def _get_nc():
    if "nc" not in _CACHE:
        _CACHE["nc"] = _build_nc()
    return _CACHE["nc"]


def _host_params(gamma, beta, fc1_w, fc1_b, dw_w, dw_b, fc2_w, fc2_b):
    bf = ml_dtypes.bfloat16
    w1g = (fc1_w * gamma[:, None]).astype(np.float32)          # [192, 384]
    s1g = w1g.sum(0)                                           # [384]
    b1aug = (beta @ fc1_w + fc1_b).astype(np.float32)          # [384]
    w1a = np.ascontiguousarray(w1g[0:128]).astype(bf)          # [128, 384]
    w1b = np.concatenate([w1g[128:192], -s1g[None, :]], 0).astype(bf)  # [65,384]
    w2 = np.ascontiguousarray(
        fc2_w.reshape(3, 128, C).transpose(1, 0, 2)).astype(bf)  # [128,3,192]
    w0, w1_, w2_ = dw_w[:, 0, 0], dw_w[:, 0, 1], dw_w[:, 0, 2]
    dwk = np.ascontiguousarray(
        dw_w[:, 0, :].reshape(3, 128, 3).transpose(1, 0, 2)).astype(np.float32)
    gbv = (dw_b + b1aug * (w0 + w1_ + w2_)).astype(np.float32)   # gelu bias
    gb = np.ascontiguousarray(gbv.reshape(3, 128).T).astype(np.float32)
    bwv = np.stack([b1aug * w0, b1aug * w2_], 1).astype(np.float32)  # [384,2]
    bw = np.ascontiguousarray(
        bwv.reshape(3, 128, 2).transpose(1, 0, 2)).astype(np.float32)
    return dict(w1a=w1a, w1b=w1b, w2=w2, dwk=dwk, gb=gb, bw=bw)


def _host_xt(x_dev):
    """Channel-major bf16 copy of x: [B, 128, 2, L]; [:, 64:, 1, :] holds the
    runtime mu row (slot 64) and padding, zero-filled here."""
    bf = ml_dtypes.bfloat16
    nb = x_dev.shape[0]
    arr = np.ascontiguousarray(
        x_dev.reshape(nb, L, C).transpose(0, 2, 1)).astype(bf)  # [nb, 192, L]
    xt = np.zeros((nb, 128, 2, L), dtype=bf)
    xt[:, :, 0, :] = arr[:, 0:128]
    xt[:, 0:64, 1, :] = arr[:, 128:192]
    return xt


def _selector_flags(x, gamma, beta, sel_w1, sel_b1, sel_w2, sel_b2):
    """Exact numpy replica of the reference direction selector. Only used
    when gamma is non-uniform (otherwise the scores tie and idx==0 always)."""
    xf = x.astype(np.float32)
    mu = xf.mean(-1, keepdims=True)
    var = ((xf - mu) ** 2).mean(-1, keepdims=True)
    xn = (xf - mu) / np.sqrt(var + EPS) * gamma + beta
    xg = xn.mean(-1)
    gh = np.abs(xg[:, :, 1:] - xg[:, :, :-1]).mean(axis=(1, 2))
    gv = np.abs(xg[:, 1:, :] - xg[:, :-1, :]).mean(axis=(1, 2))
    scores = np.stack([gh, gv, 0.8 * (gh + gv) * 0.5, np.abs(gh - gv)], 1)
    hdn = np.maximum(scores @ sel_w1 + sel_b1, 0.0)
    logits = hdn @ sel_w2 + sel_b2
    ex = np.exp(logits - logits.max(1, keepdims=True))
    probs = ex / ex.sum(1, keepdims=True)
    return probs.argmax(1) % 4 == 1


def build_in_maps(inputs):
    """Shared by kernel() and test harnesses: host preprocessing + sharding.
    Returns (in_maps, x, x_dev, flags)."""
    x = np.asarray(inputs["x"], dtype=np.float32)
    gamma = np.asarray(inputs["gamma"], np.float32)
    beta = np.asarray(inputs["beta"], np.float32)
    params = _host_params(
        gamma, beta,
        np.asarray(inputs["fc1_w"], np.float32),
        np.asarray(inputs["fc1_b"], np.float32),
        np.asarray(inputs["dw_w"], np.float32),
        np.asarray(inputs["dw_b"], np.float32),
        np.asarray(inputs["fc2_w"], np.float32),
        np.asarray(inputs["fc2_b"], np.float32),
    )

    # Routing: uniform gamma => gray image is constant => scores tie => idx 0
    # for every sample (see module docstring).  Otherwise compute the selector
    # on host and pre-transpose flagged samples (mathematically exact fixup).
    if np.ptp(gamma) == 0.0:
        flags = np.zeros(B, dtype=bool)
    else:
        flags = _selector_flags(
            x, gamma, beta,
            np.asarray(inputs["sel_w1"], np.float32),
            np.asarray(inputs["sel_b1"], np.float32),
            np.asarray(inputs["sel_w2"], np.float32),
            np.asarray(inputs["sel_b2"], np.float32))
    x_dev = x
    if flags.any():
        x_dev = x.copy()
        x_dev[flags] = np.swapaxes(x_dev[flags], 1, 2)

    xt = _host_xt(x_dev)
    xb = x_dev + np.asarray(inputs["fc2_b"], np.float32)
    in_maps = []
    for i in range(NCORES):
        m = {"x": np.ascontiguousarray(
                 x_dev[S * i:S * (i + 1)].reshape(S * L, C)),
             "xb": np.ascontiguousarray(
                 xb[S * i:S * (i + 1)].reshape(S * L, C)),
             "xt": xt[S * i:S * (i + 1)]}
        m.update(params)
        in_maps.append(m)
    return in_maps, x, x_dev, flags


def kernel(**inputs):
    from concourse.bass_utils import run_bass_kernel_spmd

    in_maps, x, x_dev, flags = build_in_maps(inputs)
    nc = _get_nc()
    res = run_bass_kernel_spmd(nc, in_maps, list(range(NCORES)))
    y = np.concatenate([r["y"].reshape(S, H, W, C) for r in res.results], 0)
    if flags.any():
        # device computed x_dev + F(x_dev); reference wants x + F(x_dev)
        # (row-major unscan orientation is identical)
        y = x + (y - x_dev)
    return y.astype(np.float32)
